# revision 45
# baseline (speedup 1.0000x reference)
"""BandSplit (nn_BandSplit_79139067396476) Trainium2 kernel.

The whole module (gather + mel-weight + per-band linear -> per-band linear +
masked scatter-add + OLA norm) is linear in x per (b, t) token. We fold it on
the host into a single banded matrix A (CF x CF, CF = C*F = 2050, interleaved
index 2*f + c) plus a constant bias vector:

    out[b, :, t] = A @ x[b, :, t] + bias

A is banded (|idx_out - idx_in| <= 2*W_max = 130), so in 128x128 block
granularity only 49 of 17x17 blocks are nonzero. The device kernel per core
(one batch element per core, tokens = T = 512 as the matmul moving dim) does:

    for each block-row r: psum[r] = sum_j Ablk[r,j]^T.T @ xT[j]  (bf16 matmul)
    out[r] = psum[r] + bias[r]  (fp32)

Everything shape/band-structure related is recomputed here with numpy
(deterministic, mirrors the reference's band construction).
"""

import numpy as np
import ml_dtypes

# ---- problem constants (hardcoded from the problem spec) ----
SR = 44100.0
N_FFT = 2048
BANDS = 128          # K
C_IN = 2             # C
D_OUT = 128
B = 8
T = 512
F = N_FFT // 2 + 1   # 1025
CF = C_IN * F        # 2050
P = 128              # partition / block size
NB = (CF + P - 1) // P   # 17 block rows/cols
CFP = NB * P         # 2176 padded
N_CORES = 8

BF16 = ml_dtypes.bfloat16


# ---- band construction (mirrors the reference implementation) ----
def _hz_to_mel(f):
    f = np.asarray(f, dtype=np.float64)
    f_sp = 200.0 / 3.0
    min_log_hz = 1000.0
    min_log_mel = min_log_hz / f_sp
    logstep = np.log(6.4) / 27.0
    lin = f / f_sp
    log = min_log_mel + np.log(np.maximum(f, 1e-10) / min_log_hz) / logstep
    return np.where(f >= min_log_hz, log, lin)


def _mel_to_hz(m):
    m = np.asarray(m, dtype=np.float64)
    f_sp = 200.0 / 3.0
    min_log_mel = 15.0
    logstep = np.log(6.4) / 27.0
    lin = m * f_sp
    log = 1000.0 * np.exp(logstep * (m - min_log_mel))
    return np.where(m >= min_log_mel, log, lin)


def _mel_filters(sr, n_fft, n_mels):
    Fq = n_fft // 2 + 1
    fftfreqs = np.linspace(0.0, sr / 2.0, Fq)
    mels = np.linspace(_hz_to_mel(0.0), _hz_to_mel(sr / 2.0), n_mels + 2)
    mel_f = _mel_to_hz(mels)
    fdiff = np.diff(mel_f)
    ramps = mel_f[:, None] - fftfreqs[None, :]
    lower = -ramps[:-2] / fdiff[:-1, None]
    upper = ramps[2:] / fdiff[1:, None]
    return np.maximum(0.0, np.minimum(lower, upper))


def _build_bands():
    mb = _mel_filters(SR, N_FFT, BANDS - 2)
    b0 = np.zeros(F)
    i = int(np.argmax(mb[0]))
    b0[:i] = 1.0 - mb[0, :i]
    bl = np.zeros(F)
    i = int(np.argmax(mb[-1]))
    bl[i:] = 1.0 - mb[-1, i:]
    melbanks = np.concatenate([b0[None, :], mb, bl[None, :]], axis=0)  # (K, F)
    ola = melbanks.sum(axis=0)
    idx_list = [np.nonzero(melbanks[k])[0] for k in range(BANDS)]
    W = max(len(ix) for ix in idx_list)
    idxs = np.full((BANDS, W), N_FFT // 2, dtype=np.int64)
    vals = np.zeros((BANDS, W), dtype=np.float64)
    for k, ix in enumerate(idx_list):
        idxs[k, : len(ix)] = ix
        vals[k, : len(ix)] = melbanks[k, ix]
    lens = np.array([len(ix) for ix in idx_list], dtype=np.int64)
    return idxs, vals, lens, ola, W


_IDXS, _MELS, _LENS, _OLA, _W = _build_bands()
_INV_OLA = (1.0 / _OLA).astype(np.float64)

# structural nonzero 128x128 blocks of A: (r, j) pairs, banded so |r - j| <= 1
_BLOCKS = []  # list of (r, j)
_seen = set()
for _k in range(BANDS):
    _fi = _IDXS[_k, : _LENS[_k]]
    lo, hi = 2 * _fi.min(), 2 * _fi.max() + 1
    for _r in range(lo // P, hi // P + 1):
        for _j in range(lo // P, hi // P + 1):
            if (_r, _j) not in _seen:
                _seen.add((_r, _j))
                _BLOCKS.append((_r, _j))
_BLOCKS.sort()
NBLK = len(_BLOCKS)
_BLK_OFF = {rj: i for i, rj in enumerate(_BLOCKS)}
_ROW_COLS = [[j for (r, j) in _BLOCKS if r == rr] for rr in range(NB)]
# blocks are (r, j)-sorted, so each row's blocks are contiguous in the pack
_ROW_START = [min((_BLK_OFF[(rr, j)] for j in _ROW_COLS[rr]), default=0)
              for rr in range(NB)]

_COMPILED = None  # (nc, ) cached Bass program


# row groups share one xt/ab load each; first group is small so the first
# matmul's inputs (incl. ~2us DMA completion latency) arrive early
_GROUP_SIZES = [2, 3, 4, 4, 3, 1]
_GROUPS = []
_row = 0
for _gs in _GROUP_SIZES:
    _GROUPS.append(list(range(_row, _row + _gs)))
    _row += _gs
assert _row == NB
_ROW_GROUP = {}
for _g, _rows in enumerate(_GROUPS):
    for _r in _rows:
        _ROW_GROUP[_r] = _g


def _build_program():
    global _COMPILED
    if _COMPILED is not None:
        return _COMPILED

    import concourse.bass as bass
    import concourse.tile as tile
    import concourse.mybir as mybir
    from concourse import bacc
    from concourse.vector_clock import ScopedClock

    class _SlimTail(tile.TileContext):
        # the stock tail (drain + EVSEM all-engine barrier + semaphore
        # clears + second barrier) costs ~8.7us on silicon; this kernel's
        # NEFF is executed once per load, so the sem cleanup is dead work.
        # Keep only the drain (it carries the waits that guarantee all
        # DMAs/compute landed).
        def _drain_and_barrier(self, tick_clock, wait_clock):
            drain_inst = self.nc.sync.drain()
            wait_clock.add_sem_waits(
                drain_inst.ins, ScopedClock({None: tick_clock.global_clock}))
            popped = self.nc._tile_sem_poison_stack.pop()
            assert popped is self._sem_poison

    import unittest.mock as mock

    # skip the ~3.4us entry all-engine butterfly emitted by Bass.__init__
    # (it only guards the const-AP memsets, which this kernel never reads)
    with mock.patch.object(bass.Bass, "all_engine_barrier",
                           lambda self, **k: None):
        nc = bacc.Bacc("TRN2", target_bir_lowering=False, debug=False,
                       num_devices=N_CORES)
    # partition-major packs: [p, r*T + t] holds row r*P+p of the logical
    # (CFP, T) matrix -> every DMA moves multi-KB contiguous runs/partition
    xt_d = nc.dram_tensor("xt", [P, NB * T], mybir.dt.bfloat16,
                          kind="ExternalInput").ap()
    ab_d = nc.dram_tensor("ab", [P, NBLK * P], mybir.dt.bfloat16,
                          kind="ExternalInput").ap()
    bias_d = nc.dram_tensor("bias", [P, NB], mybir.dt.float32,
                            kind="ExternalInput").ap()
    out_d = nc.dram_tensor("out", [P, NB * T], mybir.dt.bfloat16,
                           kind="ExternalOutput").ap()

    with _SlimTail(nc) as tc:
        with (
            tc.tile_pool(name="const", bufs=1) as cpool,
            tc.tile_pool(name="psum", bufs=8, space=bass.MemorySpace.PSUM) as ppool,
            tc.tile_pool(name="outp", bufs=3) as opool,
        ):
            bias_t = cpool.tile([P, NB], mybir.dt.float32, tag="bias")
            nc.sync.dma_start(out=bias_t[:], in_=bias_d[:])

            # prime the ACT function table while input DMAs stream
            prime_t = cpool.tile([P, 2], mybir.dt.float32, tag="prime")
            nc.gpsimd.memset(prime_t[:], 0.0)
            prime_o = cpool.tile([P, 2], mybir.dt.float32, tag="prime2")
            nc.scalar.activation(prime_o[:], prime_t[:],
                                 mybir.ActivationFunctionType.Identity,
                                 bias=0.0, scale=1.0)

            # warm the PE HAM clock gate with dummy matmuls during DMA-in
            warm_t = cpool.tile([P, T], mybir.dt.bfloat16, tag="warm")
            nc.vector.memset(warm_t[:], 0.0)
            wps = [ppool.tile([P, T], mybir.dt.float32, tag=f"warmp{i}",
                              name=f"warmp{i}", bufs=1) for i in range(2)]
            for wi in range(7):
                nc.tensor.matmul(wps[wi % 2][:], warm_t[:, 0:P], warm_t[:],
                                 start=True, stop=True)
            # all input DMAs go on Sync's single HWDGE queue, FIFO in the
            # order compute needs them — concurrent queues would fair-share
            # bandwidth and make the first-needed group finish last
            xt_tiles = {}   # group -> tile
            ab_tiles = {}   # group -> tile
            NV = CF - (NB - 1) * P   # valid partitions in the last block (2)
            for g, rows in enumerate(_GROUPS):
                r0, r1 = rows[0], rows[-1]
                xt_t = cpool.tile([P, len(rows) * T], mybir.dt.bfloat16,
                                  tag=f"xt{g}")
                # all inputs ride Sync's FIFO in need order: the single
                # queue runs at the effective HBM rate and delivers groups
                # exactly when compute needs them (a second concurrent
                # queue just fair-shares and delays the first groups)
                ld = nc.sync
                if r1 == NB - 1:
                    # last block holds only NV valid rows; zero the pad and
                    # ship just the valid partitions
                    nc.vector.memset(xt_t[:], 0.0)
                    ld.dma_start(
                        out=xt_t[0:NV, (r1 - r0) * T:(r1 - r0 + 1) * T],
                        in_=xt_d[0:NV, r1 * T:(r1 + 1) * T])
                    if r1 > r0:
                        ld.dma_start(
                            out=xt_t[:, 0:(r1 - r0) * T],
                            in_=xt_d[:, r0 * T:r1 * T])
                else:
                    ld.dma_start(out=xt_t[:],
                                 in_=xt_d[:, r0 * T:(r1 + 1) * T])
                xt_tiles[g] = xt_t
                b0 = _ROW_START[r0]
                b1 = _ROW_START[r1] + len(_ROW_COLS[r1])
                ab_t = cpool.tile([P, (b1 - b0) * P], mybir.dt.bfloat16,
                                  tag=f"ab{g}")
                ld.dma_start(out=ab_t[:], in_=ab_d[:, b0 * P:b1 * P])
                ab_tiles[g] = ab_t

            H = T // 2
            GO = 2  # rows per output store
            out_t = None
            for r in range(NB):
                g = _ROW_GROUP[r]
                cols = _ROW_COLS[r]
                psum_t = ppool.tile([P, T], mybir.dt.float32, bufs=6)
                for ci, j in enumerate(cols):
                    jg = _ROW_GROUP[j]
                    jo = j - _GROUPS[jg][0]
                    off = _BLK_OFF[(r, j)] - _ROW_START[_GROUPS[g][0]]
                    nc.tensor.matmul(
                        psum_t[:],
                        ab_tiles[g][:, off * P:(off + 1) * P],
                        xt_tiles[jg][:, jo * T:(jo + 1) * T],
                        start=(ci == 0),
                        stop=(ci == len(cols) - 1),
                    )
                # drain+bias split across Scalar and Vector halves
                if r % GO == 0:
                    n_rows = min(GO, NB - r)
                    out_t = opool.tile([P, n_rows * T], mybir.dt.bfloat16,
                                       name=f"out{r}")
                o0 = (r % GO) * T
                nc.scalar.activation(
                    out_t[:, o0:o0 + H], psum_t[:, 0:H],
                    mybir.ActivationFunctionType.Identity,
                    bias=bias_t[:, r:r + 1], scale=1.0)
                nc.vector.tensor_scalar_add(
                    out_t[:, o0 + H:o0 + T], psum_t[:, H:T],
                    bias_t[:, r:r + 1])
                if r % GO == GO - 1 or r == NB - 1:
                    r0 = (r // GO) * GO
                    n_rows = r - r0 + 1
                    # outputs ride the same Sync FIFO after the input stream:
                    # the DMA engines stay 100% busy, ins complete in need
                    # order, and out data is always ready by the time the
                    # queue drains down to it
                    # outs ride Sync's FIFO behind its (now shorter) input
                    # stream; data is always drained by the time the queue
                    # reaches it
                    if r == NB - 1 and n_rows == 1:
                        # only NV partitions of the last block are real data
                        nc.sync.dma_start(
                            out=out_d[0:NV, r0 * T:(r0 + 1) * T],
                            in_=out_t[0:NV, :])
                    else:
                        nc.sync.dma_start(
                            out=out_d[:, r0 * T:(r0 + n_rows) * T],
                            in_=out_t[:])

    nc.compile()
    _COMPILED = (nc,)
    return _COMPILED


def _host_fold(pre_w, pre_b, post_w, post_b):
    """Fold gather/mel/linears/scatter/ola into banded A blocks + bias."""
    K = BANDS
    W = _W
    # combined per-band matrix and bias (float32 is plenty: verified 6e-7)
    pre_w = np.ascontiguousarray(pre_w, dtype=np.float32)
    post_w = np.ascontiguousarray(post_w, dtype=np.float32)
    Wc = np.matmul(pre_w, post_w)                       # (K, i1, i1)
    bc = np.einsum('ko,kod->kd', pre_b.astype(np.float32), post_w) \
        + post_b.astype(np.float32)                     # (K, i1)

    A = np.zeros((CFP, CFP), dtype=np.float32)
    bias = np.zeros(CFP, dtype=np.float32)
    inv_ola = _INV_OLA.astype(np.float32)
    mels = _MELS.astype(np.float32)
    for k in range(K):
        L = int(_LENS[k])
        fi = _IDXS[k, :L]
        blk = Wc[k].reshape(W, C_IN, W, C_IN)[:L, :, :L, :]   # [w,c,wp,cp]
        blk = blk * mels[k, :L, None, None, None]
        blk = blk * inv_ola[fi][None, None, :, None]
        rows = (2 * fi[:, None] + np.arange(C_IN)[None, :]).ravel()
        A[np.ix_(rows, rows)] += blk.transpose(2, 3, 0, 1).reshape(L * C_IN,
                                                                   L * C_IN)
        bias[rows] += (bc[k].reshape(W, C_IN)[:L, :]
                       * inv_ola[fi][:, None]).ravel()

    # pack lhsT blocks: [f_in_local, f_out_local] = A[rows, cols].T
    ab = np.empty((P, NBLK * P), dtype=BF16)
    for i, (r, j) in enumerate(_BLOCKS):
        ab[:, i * P:(i + 1) * P] = A[r * P:(r + 1) * P, j * P:(j + 1) * P].T
    bias_pack = np.ascontiguousarray(bias.reshape(NB, P).T)   # (P, NB) f32
    return ab, bias_pack


def _run(inputs, trace=False, **kw):
    from concourse.bass_utils import run_bass_kernel_spmd

    x = np.asarray(inputs["x"])
    ab, bias_pack = _host_fold(
        np.asarray(inputs["pre_w"]), np.asarray(inputs["pre_b"]),
        np.asarray(inputs["post_w"]), np.asarray(inputs["post_b"]))

    (nc,) = _build_program()

    in_maps = []
    for b in range(N_CORES):
        xt = np.zeros((NB, P, T), dtype=BF16)
        # (C,T,F) -> (F,C,T) -> (F*C, T), row index = 2*f + c
        xt.reshape(CFP, T)[:CF] = x[b].transpose(2, 0, 1).reshape(CF, T)
        # partition-major pack: [p, r*T + t]
        xt_pack = np.ascontiguousarray(xt.transpose(1, 0, 2)).reshape(P, NB * T)
        in_maps.append({"xt": xt_pack, "ab": ab, "bias": bias_pack})

    res = run_bass_kernel_spmd(nc, in_maps, list(range(N_CORES)), trace=trace,
                               **kw)

    out = np.empty((B, C_IN, T, F), dtype=np.float32)
    for b in range(N_CORES):
        o = res.results[b]["out"].reshape(P, NB, T).transpose(1, 0, 2)
        o = o.reshape(CFP, T)[:CF].astype(np.float32)       # (CF, T)
        out[b] = o.reshape(F, C_IN, T).transpose(1, 2, 0)
    return out, res


def kernel(**inputs):
    out, _ = _run(inputs, trace=False)
    return out


# revision 48
# speedup vs baseline: 1.0150x; 1.0150x over previous
"""BandSplit (nn_BandSplit_79139067396476) Trainium2 kernel.

The whole module (gather + mel-weight + per-band linear -> per-band linear +
masked scatter-add + OLA norm) is linear in x per (b, t) token. We fold it on
the host into a single banded matrix A (CF x CF, CF = C*F = 2050, interleaved
index 2*f + c) plus a constant bias vector:

    out[b, :, t] = A @ x[b, :, t] + bias

A is banded (|idx_out - idx_in| <= 2*W_max = 130), so in 128x128 block
granularity only 49 of 17x17 blocks are nonzero. The device kernel per core
(one batch element per core, tokens = T = 512 as the matmul moving dim) does:

    for each block-row r: psum[r] = sum_j Ablk[r,j]^T.T @ xT[j]  (bf16 matmul)
    out[r] = psum[r] + bias[r]  (fp32)

Everything shape/band-structure related is recomputed here with numpy
(deterministic, mirrors the reference's band construction).
"""

import numpy as np
import ml_dtypes

# ---- problem constants (hardcoded from the problem spec) ----
SR = 44100.0
N_FFT = 2048
BANDS = 128          # K
C_IN = 2             # C
D_OUT = 128
B = 8
T = 512
F = N_FFT // 2 + 1   # 1025
CF = C_IN * F        # 2050
P = 128              # partition / block size
NB = (CF + P - 1) // P   # 17 block rows/cols
CFP = NB * P         # 2176 padded
N_CORES = 8

BF16 = ml_dtypes.bfloat16


# ---- band construction (mirrors the reference implementation) ----
def _hz_to_mel(f):
    f = np.asarray(f, dtype=np.float64)
    f_sp = 200.0 / 3.0
    min_log_hz = 1000.0
    min_log_mel = min_log_hz / f_sp
    logstep = np.log(6.4) / 27.0
    lin = f / f_sp
    log = min_log_mel + np.log(np.maximum(f, 1e-10) / min_log_hz) / logstep
    return np.where(f >= min_log_hz, log, lin)


def _mel_to_hz(m):
    m = np.asarray(m, dtype=np.float64)
    f_sp = 200.0 / 3.0
    min_log_mel = 15.0
    logstep = np.log(6.4) / 27.0
    lin = m * f_sp
    log = 1000.0 * np.exp(logstep * (m - min_log_mel))
    return np.where(m >= min_log_mel, log, lin)


def _mel_filters(sr, n_fft, n_mels):
    Fq = n_fft // 2 + 1
    fftfreqs = np.linspace(0.0, sr / 2.0, Fq)
    mels = np.linspace(_hz_to_mel(0.0), _hz_to_mel(sr / 2.0), n_mels + 2)
    mel_f = _mel_to_hz(mels)
    fdiff = np.diff(mel_f)
    ramps = mel_f[:, None] - fftfreqs[None, :]
    lower = -ramps[:-2] / fdiff[:-1, None]
    upper = ramps[2:] / fdiff[1:, None]
    return np.maximum(0.0, np.minimum(lower, upper))


def _build_bands():
    mb = _mel_filters(SR, N_FFT, BANDS - 2)
    b0 = np.zeros(F)
    i = int(np.argmax(mb[0]))
    b0[:i] = 1.0 - mb[0, :i]
    bl = np.zeros(F)
    i = int(np.argmax(mb[-1]))
    bl[i:] = 1.0 - mb[-1, i:]
    melbanks = np.concatenate([b0[None, :], mb, bl[None, :]], axis=0)  # (K, F)
    ola = melbanks.sum(axis=0)
    idx_list = [np.nonzero(melbanks[k])[0] for k in range(BANDS)]
    W = max(len(ix) for ix in idx_list)
    idxs = np.full((BANDS, W), N_FFT // 2, dtype=np.int64)
    vals = np.zeros((BANDS, W), dtype=np.float64)
    for k, ix in enumerate(idx_list):
        idxs[k, : len(ix)] = ix
        vals[k, : len(ix)] = melbanks[k, ix]
    lens = np.array([len(ix) for ix in idx_list], dtype=np.int64)
    return idxs, vals, lens, ola, W


_IDXS, _MELS, _LENS, _OLA, _W = _build_bands()
_INV_OLA = (1.0 / _OLA).astype(np.float64)

# structural nonzero 128x128 blocks of A: (r, j) pairs, banded so |r - j| <= 1
_BLOCKS = []  # list of (r, j)
_seen = set()
for _k in range(BANDS):
    _fi = _IDXS[_k, : _LENS[_k]]
    lo, hi = 2 * _fi.min(), 2 * _fi.max() + 1
    for _r in range(lo // P, hi // P + 1):
        for _j in range(lo // P, hi // P + 1):
            if (_r, _j) not in _seen:
                _seen.add((_r, _j))
                _BLOCKS.append((_r, _j))
_BLOCKS.sort()
NBLK = len(_BLOCKS)
_BLK_OFF = {rj: i for i, rj in enumerate(_BLOCKS)}
_ROW_COLS = [[j for (r, j) in _BLOCKS if r == rr] for rr in range(NB)]
# blocks are (r, j)-sorted, so each row's blocks are contiguous in the pack
_ROW_START = [min((_BLK_OFF[(rr, j)] for j in _ROW_COLS[rr]), default=0)
              for rr in range(NB)]

_COMPILED = None  # (nc, ) cached Bass program


# row groups share one xt/ab load each; first group is small so the first
# matmul's inputs (incl. ~2us DMA completion latency) arrive early
_GROUP_SIZES = [2, 3, 4, 4, 2, 1, 1]
_GROUPS = []
_row = 0
for _gs in _GROUP_SIZES:
    _GROUPS.append(list(range(_row, _row + _gs)))
    _row += _gs
assert _row == NB
_ROW_GROUP = {}
for _g, _rows in enumerate(_GROUPS):
    for _r in _rows:
        _ROW_GROUP[_r] = _g


def _build_program():
    global _COMPILED
    if _COMPILED is not None:
        return _COMPILED

    import concourse.bass as bass
    import concourse.tile as tile
    import concourse.mybir as mybir
    from concourse import bacc
    from concourse.vector_clock import ScopedClock

    class _SlimTail(tile.TileContext):
        # the stock tail (drain + EVSEM all-engine barrier + semaphore
        # clears + second barrier) costs ~8.7us on silicon; this kernel's
        # NEFF is executed once per load, so the sem cleanup is dead work.
        # Keep only the drain (it carries the waits that guarantee all
        # DMAs/compute landed).
        def _drain_and_barrier(self, tick_clock, wait_clock):
            drain_inst = self.nc.sync.drain()
            wait_clock.add_sem_waits(
                drain_inst.ins, ScopedClock({None: tick_clock.global_clock}))
            popped = self.nc._tile_sem_poison_stack.pop()
            assert popped is self._sem_poison

    import unittest.mock as mock

    # skip the ~3.4us entry all-engine butterfly emitted by Bass.__init__
    # (it only guards the const-AP memsets, which this kernel never reads)
    with mock.patch.object(bass.Bass, "all_engine_barrier",
                           lambda self, **k: None):
        nc = bacc.Bacc("TRN2", target_bir_lowering=False, debug=False,
                       num_devices=N_CORES)
    # partition-major packs: [p, r*T + t] holds row r*P+p of the logical
    # (CFP, T) matrix -> every DMA moves multi-KB contiguous runs/partition
    xt_d = nc.dram_tensor("xt", [P, NB * T], mybir.dt.bfloat16,
                          kind="ExternalInput").ap()
    ab_d = nc.dram_tensor("ab", [P, NBLK * P], mybir.dt.bfloat16,
                          kind="ExternalInput").ap()
    bias_d = nc.dram_tensor("bias", [P, NB], mybir.dt.float32,
                            kind="ExternalInput").ap()
    out_d = nc.dram_tensor("out", [P, NB * T], mybir.dt.bfloat16,
                           kind="ExternalOutput").ap()

    with _SlimTail(nc) as tc:
        with (
            tc.tile_pool(name="const", bufs=1) as cpool,
            tc.tile_pool(name="psum", bufs=8, space=bass.MemorySpace.PSUM) as ppool,
            tc.tile_pool(name="outp", bufs=3) as opool,
        ):
            bias_t = cpool.tile([P, NB], mybir.dt.float32, tag="bias")
            nc.sync.dma_start(out=bias_t[:], in_=bias_d[:])

            # prime the ACT function table while input DMAs stream
            prime_t = cpool.tile([P, 2], mybir.dt.float32, tag="prime")
            nc.gpsimd.memset(prime_t[:], 0.0)
            prime_o = cpool.tile([P, 2], mybir.dt.float32, tag="prime2")
            nc.scalar.activation(prime_o[:], prime_t[:],
                                 mybir.ActivationFunctionType.Identity,
                                 bias=0.0, scale=1.0)

            # warm the PE HAM clock gate with dummy matmuls during DMA-in
            warm_t = cpool.tile([P, T], mybir.dt.bfloat16, tag="warm")
            nc.vector.memset(warm_t[:], 0.0)

            # zero the pad partitions of the last xt block NOW, while the
            # Vector engine is idle — done inside the group loop it queues
            # behind the first drains and stalls Sync's dispatch FIFO
            lg = len(_GROUPS) - 1
            xt_last = cpool.tile([P, len(_GROUPS[lg]) * T],
                                 mybir.dt.bfloat16, tag=f"xt{lg}",
                                 name="xt_last")
            nc.vector.memset(xt_last[:], 0.0)
            wps = [ppool.tile([P, T], mybir.dt.float32, tag=f"warmp{i}",
                              name=f"warmp{i}", bufs=1) for i in range(2)]
            for wi in range(7):
                nc.tensor.matmul(wps[wi % 2][:], warm_t[:, 0:P], warm_t[:],
                                 start=True, stop=True)
            # all input DMAs go on Sync's single HWDGE queue, FIFO in the
            # order compute needs them — concurrent queues would fair-share
            # bandwidth and make the first-needed group finish last
            xt_tiles = {}   # group -> tile
            ab_tiles = {}   # group -> tile
            NV = CF - (NB - 1) * P   # valid partitions in the last block (2)
            for g, rows in enumerate(_GROUPS):
                r0, r1 = rows[0], rows[-1]
                # all inputs ride Sync's FIFO in need order: the single
                # queue runs at the effective HBM rate and delivers groups
                # exactly when compute needs them (a second concurrent
                # queue just fair-shares and delays the first groups)
                ld = nc.sync
                if r1 == NB - 1:
                    # last block holds only NV valid rows (pad was zeroed
                    # up front); ship just the valid partitions
                    xt_t = xt_last
                    ld.dma_start(
                        out=xt_t[0:NV, (r1 - r0) * T:(r1 - r0 + 1) * T],
                        in_=xt_d[0:NV, r1 * T:(r1 + 1) * T])
                    if r1 > r0:
                        ld.dma_start(
                            out=xt_t[:, 0:(r1 - r0) * T],
                            in_=xt_d[:, r0 * T:r1 * T])
                else:
                    xt_t = cpool.tile([P, len(rows) * T], mybir.dt.bfloat16,
                                      tag=f"xt{g}", name=f"xt_g{g}")
                    ld.dma_start(out=xt_t[:],
                                 in_=xt_d[:, r0 * T:(r1 + 1) * T])
                xt_tiles[g] = xt_t
                b0 = _ROW_START[r0]
                b1 = _ROW_START[r1] + len(_ROW_COLS[r1])
                ab_t = cpool.tile([P, (b1 - b0) * P], mybir.dt.bfloat16,
                                  tag=f"ab{g}")
                ld.dma_start(out=ab_t[:], in_=ab_d[:, b0 * P:b1 * P])
                ab_tiles[g] = ab_t

            H = T // 2
            GO = 2  # rows per output store
            out_t = None
            for r in range(NB):
                g = _ROW_GROUP[r]
                cols = _ROW_COLS[r]
                psum_t = ppool.tile([P, T], mybir.dt.float32, bufs=6)
                for ci, j in enumerate(cols):
                    jg = _ROW_GROUP[j]
                    jo = j - _GROUPS[jg][0]
                    off = _BLK_OFF[(r, j)] - _ROW_START[_GROUPS[g][0]]
                    nc.tensor.matmul(
                        psum_t[:],
                        ab_tiles[g][:, off * P:(off + 1) * P],
                        xt_tiles[jg][:, jo * T:(jo + 1) * T],
                        start=(ci == 0),
                        stop=(ci == len(cols) - 1),
                    )
                # drain+bias split across Scalar and Vector halves
                if r % GO == 0:
                    n_rows = min(GO, NB - r)
                    out_t = opool.tile([P, n_rows * T], mybir.dt.bfloat16,
                                       name=f"out{r}")
                o0 = (r % GO) * T
                nc.scalar.activation(
                    out_t[:, o0:o0 + H], psum_t[:, 0:H],
                    mybir.ActivationFunctionType.Identity,
                    bias=bias_t[:, r:r + 1], scale=1.0)
                nc.vector.tensor_scalar_add(
                    out_t[:, o0 + H:o0 + T], psum_t[:, H:T],
                    bias_t[:, r:r + 1])
                if r % GO == GO - 1 or r == NB - 1:
                    r0 = (r // GO) * GO
                    n_rows = r - r0 + 1
                    # outputs ride the same Sync FIFO after the input stream:
                    # the DMA engines stay 100% busy, ins complete in need
                    # order, and out data is always ready by the time the
                    # queue drains down to it
                    # outs ride Sync's FIFO behind its (now shorter) input
                    # stream; data is always drained by the time the queue
                    # reaches it
                    if r == NB - 1 and n_rows == 1:
                        # only NV partitions of the last block are real data
                        nc.sync.dma_start(
                            out=out_d[0:NV, r0 * T:(r0 + 1) * T],
                            in_=out_t[0:NV, :])
                    else:
                        nc.sync.dma_start(
                            out=out_d[:, r0 * T:(r0 + n_rows) * T],
                            in_=out_t[:])

    nc.compile()
    _COMPILED = (nc,)
    return _COMPILED


def _host_fold(pre_w, pre_b, post_w, post_b):
    """Fold gather/mel/linears/scatter/ola into banded A blocks + bias."""
    K = BANDS
    W = _W
    # combined per-band matrix and bias (float32 is plenty: verified 6e-7)
    pre_w = np.ascontiguousarray(pre_w, dtype=np.float32)
    post_w = np.ascontiguousarray(post_w, dtype=np.float32)
    Wc = np.matmul(pre_w, post_w)                       # (K, i1, i1)
    bc = np.einsum('ko,kod->kd', pre_b.astype(np.float32), post_w) \
        + post_b.astype(np.float32)                     # (K, i1)

    A = np.zeros((CFP, CFP), dtype=np.float32)
    bias = np.zeros(CFP, dtype=np.float32)
    inv_ola = _INV_OLA.astype(np.float32)
    mels = _MELS.astype(np.float32)
    for k in range(K):
        L = int(_LENS[k])
        fi = _IDXS[k, :L]
        blk = Wc[k].reshape(W, C_IN, W, C_IN)[:L, :, :L, :]   # [w,c,wp,cp]
        blk = blk * mels[k, :L, None, None, None]
        blk = blk * inv_ola[fi][None, None, :, None]
        rows = (2 * fi[:, None] + np.arange(C_IN)[None, :]).ravel()
        A[np.ix_(rows, rows)] += blk.transpose(2, 3, 0, 1).reshape(L * C_IN,
                                                                   L * C_IN)
        bias[rows] += (bc[k].reshape(W, C_IN)[:L, :]
                       * inv_ola[fi][:, None]).ravel()

    # pack lhsT blocks: [f_in_local, f_out_local] = A[rows, cols].T
    ab = np.empty((P, NBLK * P), dtype=BF16)
    for i, (r, j) in enumerate(_BLOCKS):
        ab[:, i * P:(i + 1) * P] = A[r * P:(r + 1) * P, j * P:(j + 1) * P].T
    bias_pack = np.ascontiguousarray(bias.reshape(NB, P).T)   # (P, NB) f32
    return ab, bias_pack


def _run(inputs, trace=False, **kw):
    from concourse.bass_utils import run_bass_kernel_spmd

    x = np.asarray(inputs["x"])
    ab, bias_pack = _host_fold(
        np.asarray(inputs["pre_w"]), np.asarray(inputs["pre_b"]),
        np.asarray(inputs["post_w"]), np.asarray(inputs["post_b"]))

    (nc,) = _build_program()

    in_maps = []
    for b in range(N_CORES):
        xt = np.zeros((NB, P, T), dtype=BF16)
        # (C,T,F) -> (F,C,T) -> (F*C, T), row index = 2*f + c
        xt.reshape(CFP, T)[:CF] = x[b].transpose(2, 0, 1).reshape(CF, T)
        # partition-major pack: [p, r*T + t]
        xt_pack = np.ascontiguousarray(xt.transpose(1, 0, 2)).reshape(P, NB * T)
        in_maps.append({"xt": xt_pack, "ab": ab, "bias": bias_pack})

    res = run_bass_kernel_spmd(nc, in_maps, list(range(N_CORES)), trace=trace,
                               **kw)

    out = np.empty((B, C_IN, T, F), dtype=np.float32)
    for b in range(N_CORES):
        o = res.results[b]["out"].reshape(P, NB, T).transpose(1, 0, 2)
        o = o.reshape(CFP, T)[:CF].astype(np.float32)       # (CF, T)
        out[b] = o.reshape(F, C_IN, T).transpose(1, 2, 0)
    return out, res


def kernel(**inputs):
    out, _ = _run(inputs, trace=False)
    return out


# revision 50
# speedup vs baseline: 1.0804x; 1.0644x over previous
"""BandSplit (nn_BandSplit_79139067396476) Trainium2 kernel.

The whole module (gather + mel-weight + per-band linear -> per-band linear +
masked scatter-add + OLA norm) is linear in x per (b, t) token. We fold it on
the host into a single banded matrix A (CF x CF, CF = C*F = 2050, interleaved
index 2*f + c) plus a constant bias vector:

    out[b, :, t] = A @ x[b, :, t] + bias

A is banded (|idx_out - idx_in| <= 2*W_max = 130), so in 128x128 block
granularity only 49 of 17x17 blocks are nonzero. The device kernel per core
(one batch element per core, tokens = T = 512 as the matmul moving dim) does:

    for each block-row r: psum[r] = sum_j Ablk[r,j]^T.T @ xT[j]  (bf16 matmul)
    out[r] = psum[r] + bias[r]  (fp32)

Everything shape/band-structure related is recomputed here with numpy
(deterministic, mirrors the reference's band construction).
"""

import numpy as np
import ml_dtypes

# ---- problem constants (hardcoded from the problem spec) ----
SR = 44100.0
N_FFT = 2048
BANDS = 128          # K
C_IN = 2             # C
D_OUT = 128
B = 8
T = 512
F = N_FFT // 2 + 1   # 1025
CF = C_IN * F        # 2050
P = 128              # partition / block size
NB = (CF + P - 1) // P   # 17 block rows/cols
CFP = NB * P         # 2176 padded
N_CORES = 8

BF16 = ml_dtypes.bfloat16


# ---- band construction (mirrors the reference implementation) ----
def _hz_to_mel(f):
    f = np.asarray(f, dtype=np.float64)
    f_sp = 200.0 / 3.0
    min_log_hz = 1000.0
    min_log_mel = min_log_hz / f_sp
    logstep = np.log(6.4) / 27.0
    lin = f / f_sp
    log = min_log_mel + np.log(np.maximum(f, 1e-10) / min_log_hz) / logstep
    return np.where(f >= min_log_hz, log, lin)


def _mel_to_hz(m):
    m = np.asarray(m, dtype=np.float64)
    f_sp = 200.0 / 3.0
    min_log_mel = 15.0
    logstep = np.log(6.4) / 27.0
    lin = m * f_sp
    log = 1000.0 * np.exp(logstep * (m - min_log_mel))
    return np.where(m >= min_log_mel, log, lin)


def _mel_filters(sr, n_fft, n_mels):
    Fq = n_fft // 2 + 1
    fftfreqs = np.linspace(0.0, sr / 2.0, Fq)
    mels = np.linspace(_hz_to_mel(0.0), _hz_to_mel(sr / 2.0), n_mels + 2)
    mel_f = _mel_to_hz(mels)
    fdiff = np.diff(mel_f)
    ramps = mel_f[:, None] - fftfreqs[None, :]
    lower = -ramps[:-2] / fdiff[:-1, None]
    upper = ramps[2:] / fdiff[1:, None]
    return np.maximum(0.0, np.minimum(lower, upper))


def _build_bands():
    mb = _mel_filters(SR, N_FFT, BANDS - 2)
    b0 = np.zeros(F)
    i = int(np.argmax(mb[0]))
    b0[:i] = 1.0 - mb[0, :i]
    bl = np.zeros(F)
    i = int(np.argmax(mb[-1]))
    bl[i:] = 1.0 - mb[-1, i:]
    melbanks = np.concatenate([b0[None, :], mb, bl[None, :]], axis=0)  # (K, F)
    ola = melbanks.sum(axis=0)
    idx_list = [np.nonzero(melbanks[k])[0] for k in range(BANDS)]
    W = max(len(ix) for ix in idx_list)
    idxs = np.full((BANDS, W), N_FFT // 2, dtype=np.int64)
    vals = np.zeros((BANDS, W), dtype=np.float64)
    for k, ix in enumerate(idx_list):
        idxs[k, : len(ix)] = ix
        vals[k, : len(ix)] = melbanks[k, ix]
    lens = np.array([len(ix) for ix in idx_list], dtype=np.int64)
    return idxs, vals, lens, ola, W


_IDXS, _MELS, _LENS, _OLA, _W = _build_bands()
_INV_OLA = (1.0 / _OLA).astype(np.float64)

# structural nonzero 128x128 blocks of A: (r, j) pairs, banded so |r - j| <= 1
_BLOCKS = []  # list of (r, j)
_seen = set()
for _k in range(BANDS):
    _fi = _IDXS[_k, : _LENS[_k]]
    lo, hi = 2 * _fi.min(), 2 * _fi.max() + 1
    for _r in range(lo // P, hi // P + 1):
        for _j in range(lo // P, hi // P + 1):
            if (_r, _j) not in _seen:
                _seen.add((_r, _j))
                _BLOCKS.append((_r, _j))
_BLOCKS.sort()
NBLK = len(_BLOCKS)
_BLK_OFF = {rj: i for i, rj in enumerate(_BLOCKS)}
_ROW_COLS = [[j for (r, j) in _BLOCKS if r == rr] for rr in range(NB)]
# blocks are (r, j)-sorted, so each row's blocks are contiguous in the pack
_ROW_START = [min((_BLK_OFF[(rr, j)] for j in _ROW_COLS[rr]), default=0)
              for rr in range(NB)]

_COMPILED = None  # (nc, ) cached Bass program


# row groups share one xt/ab load each; first group is small so the first
# matmul's inputs (incl. ~2us DMA completion latency) arrive early
_GROUP_SIZES = [2, 3, 4, 4, 2, 1, 1]
_GROUPS = []
_row = 0
for _gs in _GROUP_SIZES:
    _GROUPS.append(list(range(_row, _row + _gs)))
    _row += _gs
assert _row == NB
_ROW_GROUP = {}
for _g, _rows in enumerate(_GROUPS):
    for _r in _rows:
        _ROW_GROUP[_r] = _g


def _build_program():
    global _COMPILED
    if _COMPILED is not None:
        return _COMPILED

    import concourse.bass as bass
    import concourse.tile as tile
    import concourse.mybir as mybir
    from concourse import bacc
    from concourse.vector_clock import ScopedClock

    class _SlimTail(tile.TileContext):
        # the stock tail (drain + EVSEM all-engine barrier + semaphore
        # clears + second barrier) costs ~8.7us on silicon; this kernel's
        # NEFF is executed once per load, so the sem cleanup is dead work.
        # Keep only the drain (it carries the waits that guarantee all
        # DMAs/compute landed).
        def _drain_and_barrier(self, tick_clock, wait_clock):
            drain_inst = self.nc.sync.drain()
            wait_clock.add_sem_waits(
                drain_inst.ins, ScopedClock({None: tick_clock.global_clock}))
            popped = self.nc._tile_sem_poison_stack.pop()
            assert popped is self._sem_poison

    import unittest.mock as mock

    # skip the ~3.4us entry all-engine butterfly emitted by Bass.__init__
    # (it only guards the const-AP memsets, which this kernel never reads)
    with mock.patch.object(bass.Bass, "all_engine_barrier",
                           lambda self, **k: None):
        nc = bacc.Bacc("TRN2", target_bir_lowering=False, debug=False,
                       num_devices=N_CORES)
    # partition-major packs: [p, r*T + t] holds row r*P+p of the logical
    # (CFP, T) matrix -> every DMA moves multi-KB contiguous runs/partition
    xt_d = nc.dram_tensor("xt", [P, NB * T], mybir.dt.bfloat16,
                          kind="ExternalInput").ap()
    ab_d = nc.dram_tensor("ab", [P, NBLK * P], mybir.dt.bfloat16,
                          kind="ExternalInput").ap()
    bias_d = nc.dram_tensor("bias", [P, NB], mybir.dt.float32,
                            kind="ExternalInput").ap()
    out_d = nc.dram_tensor("out", [P, NB * T], mybir.dt.bfloat16,
                           kind="ExternalOutput").ap()

    with _SlimTail(nc) as tc:
        with (
            tc.tile_pool(name="const", bufs=1) as cpool,
            tc.tile_pool(name="psum", bufs=8, space=bass.MemorySpace.PSUM) as ppool,
            tc.tile_pool(name="outp", bufs=3) as opool,
        ):
            bias_t = cpool.tile([P, NB], mybir.dt.float32, tag="bias")
            nc.sync.dma_start(out=bias_t[:], in_=bias_d[:])

            # prime the ACT function table while input DMAs stream
            prime_t = cpool.tile([P, 2], mybir.dt.float32, tag="prime")
            nc.gpsimd.memset(prime_t[:], 0.0)
            prime_o = cpool.tile([P, 2], mybir.dt.float32, tag="prime2")
            nc.scalar.activation(prime_o[:], prime_t[:],
                                 mybir.ActivationFunctionType.Identity,
                                 bias=0.0, scale=1.0)

            # warm the PE HAM clock gate with dummy matmuls during DMA-in
            warm_t = cpool.tile([P, T], mybir.dt.bfloat16, tag="warm")
            nc.vector.memset(warm_t[:], 0.0)

            # zero the pad partitions of the last xt block NOW, while the
            # Vector engine is idle — done inside the group loop it queues
            # behind the first drains and stalls Sync's dispatch FIFO
            lg = len(_GROUPS) - 1
            xt_last = cpool.tile([P, len(_GROUPS[lg]) * T],
                                 mybir.dt.bfloat16, tag=f"xt{lg}",
                                 name="xt_last")
            nc.vector.memset(xt_last[:], 0.0)
            wps = [ppool.tile([P, T], mybir.dt.float32, tag=f"warmp{i}",
                              name=f"warmp{i}", bufs=1) for i in range(2)]
            for wi in range(11):
                nc.tensor.matmul(wps[wi % 2][:], warm_t[:, 0:P], warm_t[:],
                                 start=True, stop=True)

            def pe_filler(n):
                # dummy matmuls emitted where the real chain stalls on a
                # group's DMA-completion sem: they keep the HAM activity
                # window busy so the PE clock stays at 2.4 GHz
                for wi in range(n):
                    nc.tensor.matmul(wps[wi % 2][:], warm_t[:, 0:P],
                                     warm_t[:], start=True, stop=True)
            # all input DMAs go on Sync's single HWDGE queue, FIFO in the
            # order compute needs them — concurrent queues would fair-share
            # bandwidth and make the first-needed group finish last
            xt_tiles = {}   # group -> tile
            ab_tiles = {}   # group -> tile
            NV = CF - (NB - 1) * P   # valid partitions in the last block (2)
            for g, rows in enumerate(_GROUPS):
                r0, r1 = rows[0], rows[-1]
                # all inputs ride Sync's FIFO in need order: the single
                # queue runs at the effective HBM rate and delivers groups
                # exactly when compute needs them (a second concurrent
                # queue just fair-shares and delays the first groups)
                ld = nc.sync
                if r1 == NB - 1:
                    # last block holds only NV valid rows (pad was zeroed
                    # up front); ship just the valid partitions
                    xt_t = xt_last
                    ld.dma_start(
                        out=xt_t[0:NV, (r1 - r0) * T:(r1 - r0 + 1) * T],
                        in_=xt_d[0:NV, r1 * T:(r1 + 1) * T])
                    if r1 > r0:
                        ld.dma_start(
                            out=xt_t[:, 0:(r1 - r0) * T],
                            in_=xt_d[:, r0 * T:r1 * T])
                else:
                    xt_t = cpool.tile([P, len(rows) * T], mybir.dt.bfloat16,
                                      tag=f"xt{g}", name=f"xt_g{g}")
                    ld.dma_start(out=xt_t[:],
                                 in_=xt_d[:, r0 * T:(r1 + 1) * T])
                xt_tiles[g] = xt_t
                b0 = _ROW_START[r0]
                b1 = _ROW_START[r1] + len(_ROW_COLS[r1])
                ab_t = cpool.tile([P, (b1 - b0) * P], mybir.dt.bfloat16,
                                  tag=f"ab{g}")
                ld.dma_start(out=ab_t[:], in_=ab_d[:, b0 * P:b1 * P])
                ab_tiles[g] = ab_t

            H = T // 2
            GO = 2  # rows per output store
            out_t = None
            for r in range(NB):
                g = _ROW_GROUP[r]
                cols = _ROW_COLS[r]
                if r == _GROUPS[1][0]:
                    pe_filler(4)
                elif r == _GROUPS[2][0]:
                    pe_filler(2)
                psum_t = ppool.tile([P, T], mybir.dt.float32, bufs=6)
                for ci, j in enumerate(cols):
                    jg = _ROW_GROUP[j]
                    jo = j - _GROUPS[jg][0]
                    off = _BLK_OFF[(r, j)] - _ROW_START[_GROUPS[g][0]]
                    nc.tensor.matmul(
                        psum_t[:],
                        ab_tiles[g][:, off * P:(off + 1) * P],
                        xt_tiles[jg][:, jo * T:(jo + 1) * T],
                        start=(ci == 0),
                        stop=(ci == len(cols) - 1),
                    )
                # drain+bias split across Scalar and Vector halves
                if r % GO == 0:
                    n_rows = min(GO, NB - r)
                    out_t = opool.tile([P, n_rows * T], mybir.dt.bfloat16,
                                       name=f"out{r}")
                o0 = (r % GO) * T
                nc.scalar.activation(
                    out_t[:, o0:o0 + H], psum_t[:, 0:H],
                    mybir.ActivationFunctionType.Identity,
                    bias=bias_t[:, r:r + 1], scale=1.0)
                nc.vector.tensor_scalar_add(
                    out_t[:, o0 + H:o0 + T], psum_t[:, H:T],
                    bias_t[:, r:r + 1])
                if r % GO == GO - 1 or r == NB - 1:
                    r0 = (r // GO) * GO
                    n_rows = r - r0 + 1
                    # outputs ride the same Sync FIFO after the input stream:
                    # the DMA engines stay 100% busy, ins complete in need
                    # order, and out data is always ready by the time the
                    # queue drains down to it
                    # outs ride Sync's FIFO behind its (now shorter) input
                    # stream; data is always drained by the time the queue
                    # reaches it
                    if r == NB - 1 and n_rows == 1:
                        # only NV partitions of the last block are real data
                        nc.sync.dma_start(
                            out=out_d[0:NV, r0 * T:(r0 + 1) * T],
                            in_=out_t[0:NV, :])
                    else:
                        nc.sync.dma_start(
                            out=out_d[:, r0 * T:(r0 + n_rows) * T],
                            in_=out_t[:])

    nc.compile()
    _COMPILED = (nc,)
    return _COMPILED


def _host_fold(pre_w, pre_b, post_w, post_b):
    """Fold gather/mel/linears/scatter/ola into banded A blocks + bias."""
    K = BANDS
    W = _W
    # combined per-band matrix and bias (float32 is plenty: verified 6e-7)
    pre_w = np.ascontiguousarray(pre_w, dtype=np.float32)
    post_w = np.ascontiguousarray(post_w, dtype=np.float32)
    Wc = np.matmul(pre_w, post_w)                       # (K, i1, i1)
    bc = np.einsum('ko,kod->kd', pre_b.astype(np.float32), post_w) \
        + post_b.astype(np.float32)                     # (K, i1)

    A = np.zeros((CFP, CFP), dtype=np.float32)
    bias = np.zeros(CFP, dtype=np.float32)
    inv_ola = _INV_OLA.astype(np.float32)
    mels = _MELS.astype(np.float32)
    for k in range(K):
        L = int(_LENS[k])
        fi = _IDXS[k, :L]
        blk = Wc[k].reshape(W, C_IN, W, C_IN)[:L, :, :L, :]   # [w,c,wp,cp]
        blk = blk * mels[k, :L, None, None, None]
        blk = blk * inv_ola[fi][None, None, :, None]
        rows = (2 * fi[:, None] + np.arange(C_IN)[None, :]).ravel()
        A[np.ix_(rows, rows)] += blk.transpose(2, 3, 0, 1).reshape(L * C_IN,
                                                                   L * C_IN)
        bias[rows] += (bc[k].reshape(W, C_IN)[:L, :]
                       * inv_ola[fi][:, None]).ravel()

    # pack lhsT blocks: [f_in_local, f_out_local] = A[rows, cols].T
    ab = np.empty((P, NBLK * P), dtype=BF16)
    for i, (r, j) in enumerate(_BLOCKS):
        ab[:, i * P:(i + 1) * P] = A[r * P:(r + 1) * P, j * P:(j + 1) * P].T
    bias_pack = np.ascontiguousarray(bias.reshape(NB, P).T)   # (P, NB) f32
    return ab, bias_pack


def _run(inputs, trace=False, **kw):
    from concourse.bass_utils import run_bass_kernel_spmd

    x = np.asarray(inputs["x"])
    ab, bias_pack = _host_fold(
        np.asarray(inputs["pre_w"]), np.asarray(inputs["pre_b"]),
        np.asarray(inputs["post_w"]), np.asarray(inputs["post_b"]))

    (nc,) = _build_program()

    in_maps = []
    for b in range(N_CORES):
        xt = np.zeros((NB, P, T), dtype=BF16)
        # (C,T,F) -> (F,C,T) -> (F*C, T), row index = 2*f + c
        xt.reshape(CFP, T)[:CF] = x[b].transpose(2, 0, 1).reshape(CF, T)
        # partition-major pack: [p, r*T + t]
        xt_pack = np.ascontiguousarray(xt.transpose(1, 0, 2)).reshape(P, NB * T)
        in_maps.append({"xt": xt_pack, "ab": ab, "bias": bias_pack})

    res = run_bass_kernel_spmd(nc, in_maps, list(range(N_CORES)), trace=trace,
                               **kw)

    out = np.empty((B, C_IN, T, F), dtype=np.float32)
    for b in range(N_CORES):
        o = res.results[b]["out"].reshape(P, NB, T).transpose(1, 0, 2)
        o = o.reshape(CFP, T)[:CF].astype(np.float32)       # (CF, T)
        out[b] = o.reshape(F, C_IN, T).transpose(1, 2, 0)
    return out, res


def kernel(**inputs):
    out, _ = _run(inputs, trace=False)
    return out
